# revision 25
# baseline (speedup 1.0000x reference)
"""Trainium2 Bass kernel for nn_Attn_69801808495303.

Computes, for encoder_outputs [L, B, 2H], W [H, 2H], b [H], v [H, 1]:
    energy = tanh(enc @ W.T + b)          # [L, B, H]
    scores = energy @ v                   # [L, B]
    attn   = softmax over B (per (L, f))  # broadcast over num_features
    out    = attn as [B, num_features, L]

Strategy: shard over L across 8 NeuronCores (embarrassingly parallel —
the softmax over batch is local to every L row). Host pre-transposes the
encoder shard to [2H, L_loc*B] fp8-e4m3 (x16 scale) so the contraction
dim lands on SBUF partitions; W/b/v are replicated (W in e4m3 x512). On
device the TensorEngine runs the GEMM in fp8 DoubleRow mode (2 weights/
cell, K=256 per pass -> ~1.5-1.8x bf16 FLOP rate); ScalarE applies
tanh(psum/8192 + b) and the per-partition *v scale in fp32; VectorE
accumulates the 8 h-tiles; GpSimd reduces over partitions to finish
scores = v.tanh(...); the 64-wide batch softmax runs in quarters so it
hides under the GEMM. Each core returns its [L_loc, B] probability
block; the host concatenates and broadcasts over num_features.

fp8 numerics (validated against the fp32 reference in numpy): e4m3 on
both operands gives rel_norm ~1.7e-2 on the softmax output — under the
2e-2 gate. Scales are powers of two so dequant is exact.
"""

import sys

for _p in ("/opt/trn_rl_repo", "/opt/pypackages"):
    if _p not in sys.path:
        sys.path.append(_p)

import numpy as np
import ml_dtypes

try:  # bass_utils imports this when BASS_TRACE is set; stub so tracing
    import antenv.axon_hooks  # noqa: F401  # degrades instead of crashing
except ImportError:
    import types

    _m = types.ModuleType("antenv.axon_hooks")
    _m._hook = None
    _m.set_axon_ntff_profile_hook = lambda h: setattr(_m, "_hook", h)
    _m.get_axon_ntff_profile_hook = lambda: _m._hook
    sys.modules["antenv.axon_hooks"] = _m

L, B, H, D = 2048, 64, 1024, 2048  # D = 2H
N_CORES = 8
L_LOC = L // N_CORES        # 256 rows of L per core
M = L_LOC * B               # 16384 tokens per core
M_BLK = 512
N_BLKS = M // M_BLK         # 32
D_TILES = D // 128          # 16
H_TILES = H // 128          # 8

BF16 = ml_dtypes.bfloat16
E4M3 = ml_dtypes.float8_e4m3    # TRN FP8_EXP4: max +-240, maps to dt.float8e4
SCALE_E = 16.0                  # enc quant scale (randn -> well inside +-240)
SCALE_W = 512.0                 # W quant scale (sigma ~0.022 -> ~11)
DEQ = 1.0 / (SCALE_E * SCALE_W)  # exact power-of-two dequant inside tanh

_compiled = {}
LAST_RESULTS = None


def _build():
    import concourse.mybir as mybir
    import concourse.tile as tile
    from concourse import bacc, bass_isa

    fp32, bf16 = mybir.dt.float32, mybir.dt.bfloat16
    fp8 = mybir.dt.float8e4
    AF = mybir.ActivationFunctionType
    DR = mybir.MatmulPerfMode.DoubleRow

    nc = bacc.Bacc("TRN2", target_bir_lowering=False, debug=False,
                   num_devices=N_CORES)

    encT = nc.dram_tensor("encT", [D, M], fp8, kind="ExternalInput").ap()
    # weights pre-grouped by h-tile, contiguous per SBUF partition:
    # wr[ht, k, dt, j] = W[ht*128+j, dt*128+k]
    wr = nc.dram_tensor("wr", [H_TILES, 128, D_TILES, 128], fp8,
                        kind="ExternalInput").ap()
    bT = nc.dram_tensor("bT", [128, H_TILES], fp32, kind="ExternalInput").ap()
    vT = nc.dram_tensor("vT", [128, H_TILES], fp32, kind="ExternalInput").ap()
    out = nc.dram_tensor("out", [L_LOC, B], fp32, kind="ExternalOutput").ap()

    encT_t = encT.rearrange("(dt p) m -> p dt m", p=128)  # [128, D_TILES, M]

    with tile.TileContext(nc) as tc:
        with (
            tc.tile_pool(name="const", bufs=1) as cpool,
            tc.tile_pool(name="enc", bufs=32) as epool,
            tc.tile_pool(name="eng", bufs=4) as gpool,
            tc.tile_pool(name="veng", bufs=16) as vpool,
            tc.tile_pool(name="accp", bufs=3) as apool,
            tc.tile_pool(name="misc", bufs=2) as mpool,
            tc.tile_pool(name="psum_e", bufs=7, space="PSUM") as pe_pool,
            tc.tile_pool(name="psum_s", bufs=1, space="PSUM") as ps1pool,
            tc.tile_pool(name="dram", bufs=1, space="DRAM") as dpool,
        ):
            # Interleave the first et block's chunks with the weight DMAs so
            # the first matmuls start as soon as possible.
            wt_sb = [cpool.tile([128, D_TILES, 128], fp8, name=f"wt{ht}")
                     for ht in range(H_TILES)]

            def load_et(mb, lo=0, hi=8, chunks=None, tok0=0, ntok=M_BLK):
                msl = slice(mb * M_BLK + tok0, mb * M_BLK + tok0 + ntok)
                if chunks is None:
                    chunks = []
                for i in range(lo, hi):
                    ch = epool.tile([128, 2, ntok], fp8, tag="enc",
                                    bufs=32, name=f"et{mb}_{i}")
                    nc.sync.dma_start(ch[:], encT_t[:, 2 * i:2 * i + 2, msl])
                    chunks.append(ch)
                return chunks

            # Prologue DMA order tuned so compute never waits: et0/weight
            # tiles interleaved at the pace mb0 consumes them (every et0
            # chunk is consumed within ht0's first 1.8us, weight tile ht
            # at 11.8 + 1.8*ht us), b/v before the first activation, et1
            # behind.
            nc.sync.dma_start(wt_sb[0][:], wr[0])
            et0 = load_et(0, 0, 4)
            nc.sync.dma_start(wt_sb[1][:], wr[1])
            load_et(0, 4, 8, chunks=et0)
            nc.sync.dma_start(wt_sb[2][:], wr[2])
            b_sb = cpool.tile([128, H_TILES], fp32)
            nc.sync.dma_start(b_sb[:], bT[:])
            v_sb = cpool.tile([128, H_TILES], fp32)
            nc.sync.dma_start(v_sb[:], vT[:])
            nc.sync.dma_start(wt_sb[3][:], wr[3])
            nc.sync.dma_start(wt_sb[4][:], wr[4])
            et1 = load_et(1, 0, 2)
            for ht in range(5, H_TILES):
                nc.sync.dma_start(wt_sb[ht][:], wr[ht])
            load_et(1, 2, 8, chunks=et1)

            sc_dram = dpool.tile([1, M], fp32)

            # Warm the PE (HAM un-throttle needs ~3.4us of activity) while
            # the first weight/enc DMAs are in flight. The 4-byte DMA keeps
            # the chain alive through DCE.
            wz = cpool.tile([128, M_BLK], bf16)
            nc.gpsimd.memset(wz[:], 0.0)
            pewarm = pe_pool.tile([128, M_BLK], fp32, tag="epsum",
                                  name="pewarm")
            for i in range(10):
                nc.tensor.matmul(pewarm[:], wz[:, 0:128], wz[:],
                                 start=(i == 0), stop=(i == 9))
            warm_sb = cpool.tile([1, 1], fp32)
            nc.vector.tensor_copy(warm_sb[:], pewarm[0:1, 0:1])
            warm_dram = dpool.tile([1, 1], fp32)
            nc.sync.dma_start(warm_dram[:], warm_sb[:])

            def softmax_range(p0, p1):
                """Softmax over 64-wide batch groups for partitions
                [p0, p1) of the [128, 2, B] regrouped score view."""
                PP = p1 - p0
                sc2 = mpool.tile([PP, 2, B], fp32, tag="sc2",
                                 name=f"sc2_{p0}")
                src = sc_dram.rearrange("o (p g c) -> (o p) g c", p=128, g=2)
                nc.sync.dma_start(sc2[:], src[p0:p1])
                probs = mpool.tile([PP, 2, B], fp32, tag="probs",
                                   name=f"probs_{p0}")
                sums = mpool.tile([PP, 2], fp32, tag="sums",
                                  name=f"sums_{p0}")
                for g in range(2):
                    nc.scalar.activation(probs[:, g, :], sc2[:, g, :], AF.Exp,
                                         accum_out=sums[:, g:g + 1])
                rsum = mpool.tile([PP, 2], fp32, tag="rsum",
                                  name=f"rsum_{p0}")
                nc.vector.reciprocal(rsum[:], sums[:])
                for g in range(2):
                    nc.vector.tensor_scalar_mul(probs[:, g, :], probs[:, g, :],
                                                rsum[:, g:g + 1])
                dst = out.rearrange("(p g) c -> p g c", g=2)
                nc.sync.dma_start(dst[p0:p1], probs[:])

            def score_block(et, m0, blk, tag):
                """Energy GEMM + tanh + *v + h-sum + partition-reduce for
                tokens [m0, m0+blk); et chunk c holds d-tiles 2c, 2c+1 of
                exactly those tokens."""
                acc = apool.tile([128, blk], fp32, tag="acc",
                                 name=f"acc{tag}")
                prev_veng = None
                for ht in range(H_TILES):
                    pe = pe_pool.tile([128, blk], fp32, tag="epsum")
                    for c in range(D_TILES // 2):
                        nc.tensor.matmul(
                            pe[:], wt_sb[ht][:, 2 * c:2 * c + 2, :],
                            et[c][:, :, 0:blk],
                            start=(c == 0), stop=(c == D_TILES // 2 - 1),
                            perf_mode=DR)
                    eng = gpool.tile([128, blk], fp32, tag="eng")
                    nc.scalar.activation(eng[:], pe[:], AF.Tanh,
                                         bias=b_sb[:, ht:ht + 1], scale=DEQ)
                    veng = vpool.tile([128, blk], fp32, tag="veng",
                                      name=f"veng{tag}_{ht}")
                    nc.scalar.mul(veng[:], eng[:], v_sb[:, ht:ht + 1])
                    # running accumulation: ready ~one ACT after the last MM
                    if ht == 1:
                        nc.vector.tensor_add(acc[:], prev_veng[:], veng[:])
                    elif ht > 1:
                        nc.vector.tensor_add(acc[:], acc[:], veng[:])
                    prev_veng = veng
                # scores[m] = sum over all 1024 h = partition-reduce of acc
                red = apool.tile([128, blk], fp32, tag="red",
                                 name=f"red{tag}")
                nc.gpsimd.partition_all_reduce(red[:], acc[:], 128,
                                               bass_isa.ReduceOp.add)
                nc.sync.dma_start(sc_dram[:, m0:m0 + blk], red[0:1, :])

            v_bf = cpool.tile([128, H_TILES], bf16)
            nc.vector.tensor_copy(v_bf[:], v_sb[:])

            def tail_block(et, m0, blk):
                """Last tokens: scores via M=1 bf16 matmuls (deferred one
                h-tile so the PE never waits on ScalarE) and an inline
                single-partition softmax — a much shorter critical chain
                than the gpsimd/DRAM-bounce path."""
                nl = blk // B  # l rows covered
                sps = ps1pool.tile([1, blk], fp32, tag="sps")
                engs = []
                for ht in range(H_TILES):
                    pe = pe_pool.tile([128, blk], fp32, tag="epsum")
                    for c in range(D_TILES // 2):
                        nc.tensor.matmul(
                            pe[:], wt_sb[ht][:, 2 * c:2 * c + 2, :],
                            et[c][:, :, 0:blk],
                            start=(c == 0), stop=(c == D_TILES // 2 - 1),
                            perf_mode=DR)
                    eng = gpool.tile([128, blk], bf16, tag="engbf",
                                     name=f"engbf{ht}")
                    nc.scalar.activation(eng[:], pe[:], AF.Tanh,
                                         bias=b_sb[:, ht:ht + 1], scale=DEQ)
                    engs.append(eng)
                    # defer the score matvec two h-tiles so it never waits
                    # on the ScalarE queue
                    if ht >= 2:
                        nc.tensor.matmul(sps[:], v_bf[:, ht - 2:ht - 1],
                                         engs[ht - 2][:], start=(ht == 2),
                                         stop=False)
                for ht in (H_TILES - 2, H_TILES - 1):
                    nc.tensor.matmul(sps[:], v_bf[:, ht:ht + 1],
                                     engs[ht][:], start=False,
                                     stop=(ht == H_TILES - 1))
                st = mpool.tile([1, nl, B], fp32, tag="st")
                nc.scalar.activation(st[:], sps.rearrange("o (l c) -> o l c",
                                                          c=B), AF.Exp)
                tsum = mpool.tile([1, nl], fp32, tag="tsum")
                nc.vector.reduce_sum(tsum[:], st[:],
                                     axis=mybir.AxisListType.X)
                trs = mpool.tile([1, nl], fp32, tag="trs")
                nc.vector.reciprocal(trs[:], tsum[:])
                nc.vector.tensor_tensor(st[:], st[:],
                                        trs[:, :, None].to_broadcast(st.shape),
                                        mybir.AluOpType.mult)
                l0 = m0 // B
                dst = out.rearrange("(a l) c -> a l c", l=nl)
                nc.sync.dma_start(dst[l0 // nl:l0 // nl + 1], st[:])

            # Partitions 96..123 normalize after mb30 (covered by mb31's
            # score half + the tail block); only the tiny (124,126) range
            # waits on the last gpsimd/DRAM bounce, hidden under the tail
            # block's matmuls. The kernel ends on the short-chain tail.
            TAIL = 256
            for mb in range(N_BLKS - 1):
                et = (et0 if mb == 0 else
                      et1 if mb == 1 else load_et(mb))
                score_block(et, mb * M_BLK, M_BLK, str(mb))
                if mb == 7:
                    softmax_range(0, 32)
                elif mb == 15:
                    softmax_range(32, 64)
                elif mb == 23:
                    softmax_range(64, 96)
                elif mb == 30:
                    softmax_range(96, 124)
            et31a = load_et(31, tok0=0, ntok=M_BLK - TAIL)
            score_block(et31a, 31 * M_BLK, M_BLK - TAIL, "31a")
            softmax_range(124, 126)
            et31b = load_et(31, tok0=M_BLK - TAIL, ntok=TAIL)
            tail_block(et31b, 31 * M_BLK + (M_BLK - TAIL), TAIL)

    nc.compile()
    return nc


def kernel(num_features, encoder_outputs, W, b, v):
    global LAST_RESULTS
    from concourse.bass_utils import run_bass_kernel_spmd

    enc = np.asarray(encoder_outputs, dtype=np.float32)
    W_np = np.asarray(W, dtype=np.float32)
    b_np = np.asarray(b, dtype=np.float32)
    v_np = np.asarray(v, dtype=np.float32)
    F = int(np.asarray(num_features))
    assert enc.shape == (L, B, D) and W_np.shape == (H, D)

    # wr[ht, k, dt, j] = W[ht*128 + j, dt*128 + k], quantized to e4m3 at
    # x512 — contiguous 2KB per SBUF partition for a single clean DMA.
    wr_np = np.clip(
        W_np.reshape(H_TILES, 128, D_TILES, 128).transpose(0, 3, 2, 1)
        * SCALE_W, -240.0, 240.0).astype(E4M3)
    wr_np = np.ascontiguousarray(wr_np)
    bT_np = np.ascontiguousarray(b_np.reshape(H_TILES, 128).T)     # [128, 8]
    vT_np = np.ascontiguousarray(v_np.ravel().reshape(H_TILES, 128).T)

    in_maps = []
    for c in range(N_CORES):
        shard = np.clip(
            enc[c * L_LOC:(c + 1) * L_LOC].reshape(M, D) * SCALE_E,
            -240.0, 240.0).astype(E4M3)
        encT_np = np.ascontiguousarray(shard.T)                    # [D, M]
        in_maps.append({"encT": encT_np, "wr": wr_np, "bT": bT_np,
                        "vT": vT_np})

    if "nc" not in _compiled:
        _compiled["nc"] = _build()
    nc = _compiled["nc"]

    res = run_bass_kernel_spmd(nc, in_maps, core_ids=list(range(N_CORES)))
    LAST_RESULTS = res

    probs = np.concatenate([res.results[c]["out"] for c in range(N_CORES)],
                           axis=0)                                 # [L, B]
    out = np.broadcast_to(probs.T[:, None, :], (B, F, L))
    return np.ascontiguousarray(out)



# revision 27
# speedup vs baseline: 1.0049x; 1.0049x over previous
"""Trainium2 Bass kernel for nn_Attn_69801808495303.

Computes, for encoder_outputs [L, B, 2H], W [H, 2H], b [H], v [H, 1]:
    energy = tanh(enc @ W.T + b)          # [L, B, H]
    scores = energy @ v                   # [L, B]
    attn   = softmax over B (per (L, f))  # broadcast over num_features
    out    = attn as [B, num_features, L]

Strategy: shard over L across 8 NeuronCores (embarrassingly parallel —
the softmax over batch is local to every L row). Host pre-transposes the
encoder shard to [2H, L_loc*B] fp8-e4m3 (x16 scale) so the contraction
dim lands on SBUF partitions; W/b/v are replicated (W in e4m3 x512). On
device the TensorEngine runs the GEMM in fp8 DoubleRow mode (2 weights/
cell, K=256 per pass -> ~1.5-1.8x bf16 FLOP rate); ScalarE applies
tanh(psum/8192 + b) and the per-partition *v scale in fp32; VectorE
accumulates the 8 h-tiles; GpSimd reduces over partitions to finish
scores = v.tanh(...); the 64-wide batch softmax runs in quarters so it
hides under the GEMM. Each core returns its [L_loc, B] probability
block; the host concatenates and broadcasts over num_features.

fp8 numerics (validated against the fp32 reference in numpy): e4m3 on
both operands gives rel_norm ~1.7e-2 on the softmax output — under the
2e-2 gate. Scales are powers of two so dequant is exact.
"""

import sys

for _p in ("/opt/trn_rl_repo", "/opt/pypackages"):
    if _p not in sys.path:
        sys.path.append(_p)

import numpy as np
import ml_dtypes

try:  # bass_utils imports this when BASS_TRACE is set; stub so tracing
    import antenv.axon_hooks  # noqa: F401  # degrades instead of crashing
except ImportError:
    import types

    _m = types.ModuleType("antenv.axon_hooks")
    _m._hook = None
    _m.set_axon_ntff_profile_hook = lambda h: setattr(_m, "_hook", h)
    _m.get_axon_ntff_profile_hook = lambda: _m._hook
    sys.modules["antenv.axon_hooks"] = _m

L, B, H, D = 2048, 64, 1024, 2048  # D = 2H
N_CORES = 8
L_LOC = L // N_CORES        # 256 rows of L per core
M = L_LOC * B               # 16384 tokens per core
M_BLK = 512
N_BLKS = M // M_BLK         # 32
D_TILES = D // 128          # 16
H_TILES = H // 128          # 8

BF16 = ml_dtypes.bfloat16
E4M3 = ml_dtypes.float8_e4m3    # TRN FP8_EXP4: max +-240, maps to dt.float8e4
SCALE_E = 16.0                  # enc quant scale (randn -> well inside +-240)
SCALE_W = 512.0                 # W quant scale (sigma ~0.022 -> ~11)
DEQ = 1.0 / (SCALE_E * SCALE_W)  # exact power-of-two dequant inside tanh

_compiled = {}
LAST_RESULTS = None


def _build():
    import concourse.mybir as mybir
    import concourse.tile as tile
    from concourse import bacc, bass_isa

    fp32, bf16 = mybir.dt.float32, mybir.dt.bfloat16
    fp8 = mybir.dt.float8e4
    AF = mybir.ActivationFunctionType
    DR = mybir.MatmulPerfMode.DoubleRow

    nc = bacc.Bacc("TRN2", target_bir_lowering=False, debug=False,
                   num_devices=N_CORES)

    encT = nc.dram_tensor("encT", [D, M], fp8, kind="ExternalInput").ap()
    # weights pre-grouped by h-tile, contiguous per SBUF partition:
    # wr[ht, k, dt, j] = W[ht*128+j, dt*128+k]
    wr = nc.dram_tensor("wr", [H_TILES, 128, D_TILES, 128], fp8,
                        kind="ExternalInput").ap()
    bT = nc.dram_tensor("bT", [128, H_TILES], fp32, kind="ExternalInput").ap()
    vT = nc.dram_tensor("vT", [128, H_TILES], fp32, kind="ExternalInput").ap()
    out = nc.dram_tensor("out", [L_LOC, B], fp32, kind="ExternalOutput").ap()

    encT_t = encT.rearrange("(dt p) m -> p dt m", p=128)  # [128, D_TILES, M]

    with tile.TileContext(nc) as tc:
        with (
            tc.tile_pool(name="const", bufs=1) as cpool,
            tc.tile_pool(name="enc", bufs=32) as epool,
            tc.tile_pool(name="eng", bufs=4) as gpool,
            tc.tile_pool(name="veng", bufs=16) as vpool,
            tc.tile_pool(name="accp", bufs=3) as apool,
            tc.tile_pool(name="misc", bufs=2) as mpool,
            tc.tile_pool(name="psum_e", bufs=7, space="PSUM") as pe_pool,
            tc.tile_pool(name="psum_s", bufs=1, space="PSUM") as ps1pool,
            tc.tile_pool(name="dram", bufs=1, space="DRAM") as dpool,
        ):
            # Interleave the first et block's chunks with the weight DMAs so
            # the first matmuls start as soon as possible.
            wt_sb = [cpool.tile([128, D_TILES, 128], fp8, name=f"wt{ht}")
                     for ht in range(H_TILES)]

            def load_et(mb, lo=0, hi=8, chunks=None, tok0=0, ntok=M_BLK,
                        eng=None):
                msl = slice(mb * M_BLK + tok0, mb * M_BLK + tok0 + ntok)
                if chunks is None:
                    chunks = []
                for i in range(lo, hi):
                    ch = epool.tile([128, 2, ntok], fp8, tag="enc",
                                    bufs=32, name=f"et{mb}_{i}")
                    (eng or nc.sync).dma_start(
                        ch[:], encT_t[:, 2 * i:2 * i + 2, msl])
                    chunks.append(ch)
                return chunks

            # Prologue DMAs fan out over the idle scalar/vector/gpsimd
            # queues so the serial ~0.7us-per-DMA issue cost on one queue
            # never paces the start: et0 lands by ~8.5us, weights stream
            # on sync at the pace mb0 consumes them.
            wz = cpool.tile([128, M_BLK], bf16)
            nc.gpsimd.memset(wz[:], 0.0)
            nc.sync.dma_start(wt_sb[0][:], wr[0])
            et0 = load_et(0, 0, 4, eng=nc.scalar)
            load_et(0, 4, 8, chunks=et0, eng=nc.gpsimd)
            b_sb = cpool.tile([128, H_TILES], fp32)
            nc.scalar.dma_start(b_sb[:], bT[:])
            v_sb = cpool.tile([128, H_TILES], fp32)
            nc.scalar.dma_start(v_sb[:], vT[:])
            for ht in range(1, H_TILES):
                nc.sync.dma_start(wt_sb[ht][:], wr[ht])
            et1 = load_et(1, 0, 3, eng=nc.gpsimd)
            load_et(1, 3, 6, chunks=et1, eng=nc.scalar)
            load_et(1, 6, 8, chunks=et1)

            sc_dram = dpool.tile([1, M], fp32)

            # Warm the PE (HAM un-throttle needs ~3.4us of activity) while
            # the first weight/enc DMAs are in flight. The 4-byte DMA keeps
            # the chain alive through DCE.
            pewarm = pe_pool.tile([128, M_BLK], fp32, tag="epsum",
                                  name="pewarm")
            for i in range(8):
                nc.tensor.matmul(pewarm[:], wz[:, 0:128], wz[:],
                                 start=(i == 0), stop=(i == 7))
            warm_sb = cpool.tile([1, 1], fp32)
            nc.vector.tensor_copy(warm_sb[:], pewarm[0:1, 0:1])
            warm_dram = dpool.tile([1, 1], fp32)
            nc.sync.dma_start(warm_dram[:], warm_sb[:])

            def softmax_range(p0, p1):
                """Softmax over 64-wide batch groups for partitions
                [p0, p1) of the [128, 2, B] regrouped score view."""
                PP = p1 - p0
                sc2 = mpool.tile([PP, 2, B], fp32, tag="sc2",
                                 name=f"sc2_{p0}")
                src = sc_dram.rearrange("o (p g c) -> (o p) g c", p=128, g=2)
                nc.sync.dma_start(sc2[:], src[p0:p1])
                probs = mpool.tile([PP, 2, B], fp32, tag="probs",
                                   name=f"probs_{p0}")
                sums = mpool.tile([PP, 2], fp32, tag="sums",
                                  name=f"sums_{p0}")
                for g in range(2):
                    nc.scalar.activation(probs[:, g, :], sc2[:, g, :], AF.Exp,
                                         accum_out=sums[:, g:g + 1])
                rsum = mpool.tile([PP, 2], fp32, tag="rsum",
                                  name=f"rsum_{p0}")
                nc.vector.reciprocal(rsum[:], sums[:])
                for g in range(2):
                    nc.vector.tensor_scalar_mul(probs[:, g, :], probs[:, g, :],
                                                rsum[:, g:g + 1])
                dst = out.rearrange("(p g) c -> p g c", g=2)
                nc.sync.dma_start(dst[p0:p1], probs[:])

            def score_block(et, m0, blk, tag):
                """Energy GEMM + tanh + *v + h-sum + partition-reduce for
                tokens [m0, m0+blk); et chunk c holds d-tiles 2c, 2c+1 of
                exactly those tokens."""
                acc = apool.tile([128, blk], fp32, tag="acc",
                                 name=f"acc{tag}")
                prev_veng = None
                for ht in range(H_TILES):
                    pe = pe_pool.tile([128, blk], fp32, tag="epsum")
                    for c in range(D_TILES // 2):
                        nc.tensor.matmul(
                            pe[:], wt_sb[ht][:, 2 * c:2 * c + 2, :],
                            et[c][:, :, 0:blk],
                            start=(c == 0), stop=(c == D_TILES // 2 - 1),
                            perf_mode=DR)
                    eng = gpool.tile([128, blk], fp32, tag="eng")
                    nc.scalar.activation(eng[:], pe[:], AF.Tanh,
                                         bias=b_sb[:, ht:ht + 1], scale=DEQ)
                    veng = vpool.tile([128, blk], fp32, tag="veng",
                                      name=f"veng{tag}_{ht}")
                    nc.scalar.mul(veng[:], eng[:], v_sb[:, ht:ht + 1])
                    # running accumulation: ready ~one ACT after the last MM
                    if ht == 1:
                        nc.vector.tensor_add(acc[:], prev_veng[:], veng[:])
                    elif ht > 1:
                        nc.vector.tensor_add(acc[:], acc[:], veng[:])
                    prev_veng = veng
                # scores[m] = sum over all 1024 h = partition-reduce of acc
                red = apool.tile([128, blk], fp32, tag="red",
                                 name=f"red{tag}")
                nc.gpsimd.partition_all_reduce(red[:], acc[:], 128,
                                               bass_isa.ReduceOp.add)
                nc.sync.dma_start(sc_dram[:, m0:m0 + blk], red[0:1, :])

            v_bf = cpool.tile([128, H_TILES], bf16)
            nc.vector.tensor_copy(v_bf[:], v_sb[:])

            def tail_block(et, m0, blk):
                """Last tokens: scores via M=1 bf16 matmuls (deferred one
                h-tile so the PE never waits on ScalarE) and an inline
                single-partition softmax — a much shorter critical chain
                than the gpsimd/DRAM-bounce path."""
                nl = blk // B  # l rows covered
                sps = ps1pool.tile([1, blk], fp32, tag="sps")
                engs = []
                for ht in range(H_TILES):
                    pe = pe_pool.tile([128, blk], fp32, tag="epsum")
                    for c in range(D_TILES // 2):
                        nc.tensor.matmul(
                            pe[:], wt_sb[ht][:, 2 * c:2 * c + 2, :],
                            et[c][:, :, 0:blk],
                            start=(c == 0), stop=(c == D_TILES // 2 - 1),
                            perf_mode=DR)
                    eng = gpool.tile([128, blk], bf16, tag="engbf",
                                     name=f"engbf{ht}")
                    nc.scalar.activation(eng[:], pe[:], AF.Tanh,
                                         bias=b_sb[:, ht:ht + 1], scale=DEQ)
                    engs.append(eng)
                    # defer the score matvec two h-tiles so it never waits
                    # on the ScalarE queue
                    if ht >= 2:
                        nc.tensor.matmul(sps[:], v_bf[:, ht - 2:ht - 1],
                                         engs[ht - 2][:], start=(ht == 2),
                                         stop=False)
                for ht in (H_TILES - 2, H_TILES - 1):
                    nc.tensor.matmul(sps[:], v_bf[:, ht:ht + 1],
                                     engs[ht][:], start=False,
                                     stop=(ht == H_TILES - 1))
                st = mpool.tile([1, nl, B], fp32, tag="st")
                nc.scalar.activation(st[:], sps.rearrange("o (l c) -> o l c",
                                                          c=B), AF.Exp)
                tsum = mpool.tile([1, nl], fp32, tag="tsum")
                nc.vector.reduce_sum(tsum[:], st[:],
                                     axis=mybir.AxisListType.X)
                trs = mpool.tile([1, nl], fp32, tag="trs")
                nc.vector.reciprocal(trs[:], tsum[:])
                nc.vector.tensor_tensor(st[:], st[:],
                                        trs[:, :, None].to_broadcast(st.shape),
                                        mybir.AluOpType.mult)
                l0 = m0 // B
                dst = out.rearrange("(a l) c -> a l c", l=nl)
                nc.sync.dma_start(dst[l0 // nl:l0 // nl + 1], st[:])

            # Partitions 96..123 normalize after mb30 (covered by mb31's
            # score half + the tail block); only the tiny (124,126) range
            # waits on the last gpsimd/DRAM bounce, hidden under the tail
            # block's matmuls. The kernel ends on the short-chain tail.
            TAIL = 256
            for mb in range(N_BLKS - 1):
                et = (et0 if mb == 0 else
                      et1 if mb == 1 else load_et(mb))
                score_block(et, mb * M_BLK, M_BLK, str(mb))
                if mb == 7:
                    softmax_range(0, 32)
                elif mb == 15:
                    softmax_range(32, 64)
                elif mb == 23:
                    softmax_range(64, 96)
                elif mb == 30:
                    softmax_range(96, 124)
            et31a = load_et(31, tok0=0, ntok=M_BLK - TAIL)
            score_block(et31a, 31 * M_BLK, M_BLK - TAIL, "31a")
            softmax_range(124, 126)
            et31b = load_et(31, tok0=M_BLK - TAIL, ntok=TAIL)
            tail_block(et31b, 31 * M_BLK + (M_BLK - TAIL), TAIL)

    nc.compile()
    return nc


def kernel(num_features, encoder_outputs, W, b, v):
    global LAST_RESULTS
    from concourse.bass_utils import run_bass_kernel_spmd

    enc = np.asarray(encoder_outputs, dtype=np.float32)
    W_np = np.asarray(W, dtype=np.float32)
    b_np = np.asarray(b, dtype=np.float32)
    v_np = np.asarray(v, dtype=np.float32)
    F = int(np.asarray(num_features))
    assert enc.shape == (L, B, D) and W_np.shape == (H, D)

    # wr[ht, k, dt, j] = W[ht*128 + j, dt*128 + k], quantized to e4m3 at
    # x512 — contiguous 2KB per SBUF partition for a single clean DMA.
    wr_np = np.clip(
        W_np.reshape(H_TILES, 128, D_TILES, 128).transpose(0, 3, 2, 1)
        * SCALE_W, -240.0, 240.0).astype(E4M3)
    wr_np = np.ascontiguousarray(wr_np)
    bT_np = np.ascontiguousarray(b_np.reshape(H_TILES, 128).T)     # [128, 8]
    vT_np = np.ascontiguousarray(v_np.ravel().reshape(H_TILES, 128).T)

    in_maps = []
    for c in range(N_CORES):
        shard = np.clip(
            enc[c * L_LOC:(c + 1) * L_LOC].reshape(M, D) * SCALE_E,
            -240.0, 240.0).astype(E4M3)
        encT_np = np.ascontiguousarray(shard.T)                    # [D, M]
        in_maps.append({"encT": encT_np, "wr": wr_np, "bT": bT_np,
                        "vT": vT_np})

    if "nc" not in _compiled:
        _compiled["nc"] = _build()
    nc = _compiled["nc"]

    res = run_bass_kernel_spmd(nc, in_maps, core_ids=list(range(N_CORES)))
    LAST_RESULTS = res

    probs = np.concatenate([res.results[c]["out"] for c in range(N_CORES)],
                           axis=0)                                 # [L, B]
    out = np.broadcast_to(probs.T[:, None, :], (B, F, L))
    return np.ascontiguousarray(out)



# revision 28
# speedup vs baseline: 1.0085x; 1.0036x over previous
"""Trainium2 Bass kernel for nn_Attn_69801808495303.

Computes, for encoder_outputs [L, B, 2H], W [H, 2H], b [H], v [H, 1]:
    energy = tanh(enc @ W.T + b)          # [L, B, H]
    scores = energy @ v                   # [L, B]
    attn   = softmax over B (per (L, f))  # broadcast over num_features
    out    = attn as [B, num_features, L]

Strategy: shard over L across 8 NeuronCores (embarrassingly parallel —
the softmax over batch is local to every L row). Host pre-transposes the
encoder shard to [2H, L_loc*B] fp8-e4m3 (x16 scale) so the contraction
dim lands on SBUF partitions; W/b/v are replicated (W in e4m3 x512). On
device the TensorEngine runs the GEMM in fp8 DoubleRow mode (2 weights/
cell, K=256 per pass -> ~1.5-1.8x bf16 FLOP rate); ScalarE applies
tanh(psum/8192 + b) and the per-partition *v scale in fp32; VectorE
accumulates the 8 h-tiles; GpSimd reduces over partitions to finish
scores = v.tanh(...); the 64-wide batch softmax runs in quarters so it
hides under the GEMM. Each core returns its [L_loc, B] probability
block; the host concatenates and broadcasts over num_features.

fp8 numerics (validated against the fp32 reference in numpy): e4m3 on
both operands gives rel_norm ~1.7e-2 on the softmax output — under the
2e-2 gate. Scales are powers of two so dequant is exact.
"""

import sys

for _p in ("/opt/trn_rl_repo", "/opt/pypackages"):
    if _p not in sys.path:
        sys.path.append(_p)

import numpy as np
import ml_dtypes

try:  # bass_utils imports this when BASS_TRACE is set; stub so tracing
    import antenv.axon_hooks  # noqa: F401  # degrades instead of crashing
except ImportError:
    import types

    _m = types.ModuleType("antenv.axon_hooks")
    _m._hook = None
    _m.set_axon_ntff_profile_hook = lambda h: setattr(_m, "_hook", h)
    _m.get_axon_ntff_profile_hook = lambda: _m._hook
    sys.modules["antenv.axon_hooks"] = _m

L, B, H, D = 2048, 64, 1024, 2048  # D = 2H
N_CORES = 8
L_LOC = L // N_CORES        # 256 rows of L per core
M = L_LOC * B               # 16384 tokens per core
M_BLK = 512
N_BLKS = M // M_BLK         # 32
D_TILES = D // 128          # 16
H_TILES = H // 128          # 8

BF16 = ml_dtypes.bfloat16
E4M3 = ml_dtypes.float8_e4m3    # TRN FP8_EXP4: max +-240, maps to dt.float8e4
SCALE_E = 16.0                  # enc quant scale (randn -> well inside +-240)
SCALE_W = 512.0                 # W quant scale (sigma ~0.022 -> ~11)
DEQ = 1.0 / (SCALE_E * SCALE_W)  # exact power-of-two dequant inside tanh

_compiled = {}
LAST_RESULTS = None


def _build():
    import concourse.mybir as mybir
    import concourse.tile as tile
    from concourse import bacc, bass_isa

    fp32, bf16 = mybir.dt.float32, mybir.dt.bfloat16
    fp8 = mybir.dt.float8e4
    AF = mybir.ActivationFunctionType
    DR = mybir.MatmulPerfMode.DoubleRow

    nc = bacc.Bacc("TRN2", target_bir_lowering=False, debug=False,
                   num_devices=N_CORES)

    encT = nc.dram_tensor("encT", [D, M], fp8, kind="ExternalInput").ap()
    # weights pre-grouped by h-tile, contiguous per SBUF partition:
    # wr[ht, k, dt, j] = W[ht*128+j, dt*128+k]
    wr = nc.dram_tensor("wr", [H_TILES, 128, D_TILES, 128], fp8,
                        kind="ExternalInput").ap()
    bT = nc.dram_tensor("bT", [128, H_TILES], fp32, kind="ExternalInput").ap()
    vT = nc.dram_tensor("vT", [128, H_TILES], fp32, kind="ExternalInput").ap()
    out = nc.dram_tensor("out", [L_LOC, B], fp32, kind="ExternalOutput").ap()

    encT_t = encT.rearrange("(dt p) m -> p dt m", p=128)  # [128, D_TILES, M]

    with tile.TileContext(nc) as tc:
        with (
            tc.tile_pool(name="const", bufs=1) as cpool,
            tc.tile_pool(name="enc", bufs=32) as epool,
            tc.tile_pool(name="eng", bufs=4) as gpool,
            tc.tile_pool(name="veng", bufs=16) as vpool,
            tc.tile_pool(name="accp", bufs=3) as apool,
            tc.tile_pool(name="misc", bufs=2) as mpool,
            tc.tile_pool(name="psum_e", bufs=7, space="PSUM") as pe_pool,
            tc.tile_pool(name="psum_s", bufs=1, space="PSUM") as ps1pool,
            tc.tile_pool(name="dram", bufs=1, space="DRAM") as dpool,
        ):
            # Interleave the first et block's chunks with the weight DMAs so
            # the first matmuls start as soon as possible.
            wt_sb = [cpool.tile([128, D_TILES, 128], fp8, name=f"wt{ht}")
                     for ht in range(H_TILES)]

            def load_et(mb, lo=0, hi=8, chunks=None, tok0=0, ntok=M_BLK,
                        eng=None):
                msl = slice(mb * M_BLK + tok0, mb * M_BLK + tok0 + ntok)
                if chunks is None:
                    chunks = []
                for i in range(lo, hi):
                    ch = epool.tile([128, 2, ntok], fp8, tag="enc",
                                    bufs=32, name=f"et{mb}_{i}")
                    (eng or nc.sync).dma_start(
                        ch[:], encT_t[:, 2 * i:2 * i + 2, msl])
                    chunks.append(ch)
                return chunks

            # Prologue DMA order tuned so compute never waits: et0/weight
            # tiles interleaved at the pace mb0 consumes them (every et0
            # chunk is consumed within ht0's first 1.8us, weight tile ht
            # at 11.8 + 1.8*ht us), b/v before the first activation, et1
            # behind. (All on the sync queue: scalar/gpsimd DMA issue
            # measured slower.)
            nc.sync.dma_start(wt_sb[0][:], wr[0])
            et0 = load_et(0, 0, 4)
            nc.sync.dma_start(wt_sb[1][:], wr[1])
            load_et(0, 4, 8, chunks=et0)
            nc.sync.dma_start(wt_sb[2][:], wr[2])
            b_sb = cpool.tile([128, H_TILES], fp32)
            nc.sync.dma_start(b_sb[:], bT[:])
            v_sb = cpool.tile([128, H_TILES], fp32)
            nc.sync.dma_start(v_sb[:], vT[:])
            nc.sync.dma_start(wt_sb[3][:], wr[3])
            nc.sync.dma_start(wt_sb[4][:], wr[4])
            et1 = load_et(1, 0, 2)
            for ht in range(5, H_TILES):
                nc.sync.dma_start(wt_sb[ht][:], wr[ht])
            load_et(1, 2, 8, chunks=et1)

            sc_dram = dpool.tile([1, M], fp32)

            # Warm the PE (HAM un-throttle needs ~3.4us of activity) while
            # the first weight/enc DMAs are in flight. The 4-byte DMA keeps
            # the chain alive through DCE.
            wz = cpool.tile([128, M_BLK], bf16)
            nc.gpsimd.memset(wz[:], 0.0)
            pewarm = pe_pool.tile([128, M_BLK], fp32, tag="epsum",
                                  name="pewarm")
            for i in range(10):
                nc.tensor.matmul(pewarm[:], wz[:, 0:128], wz[:],
                                 start=(i == 0), stop=(i == 9))
            warm_sb = cpool.tile([1, 1], fp32)
            nc.vector.tensor_copy(warm_sb[:], pewarm[0:1, 0:1])
            warm_dram = dpool.tile([1, 1], fp32)
            nc.sync.dma_start(warm_dram[:], warm_sb[:])

            def softmax_range(p0, p1):
                """Softmax over 64-wide batch groups for partitions
                [p0, p1) of the [128, 2, B] regrouped score view."""
                PP = p1 - p0
                sc2 = mpool.tile([PP, 2, B], fp32, tag="sc2",
                                 name=f"sc2_{p0}")
                src = sc_dram.rearrange("o (p g c) -> (o p) g c", p=128, g=2)
                nc.sync.dma_start(sc2[:], src[p0:p1])
                probs = mpool.tile([PP, 2, B], fp32, tag="probs",
                                   name=f"probs_{p0}")
                sums = mpool.tile([PP, 2], fp32, tag="sums",
                                  name=f"sums_{p0}")
                for g in range(2):
                    nc.scalar.activation(probs[:, g, :], sc2[:, g, :], AF.Exp,
                                         accum_out=sums[:, g:g + 1])
                rsum = mpool.tile([PP, 2], fp32, tag="rsum",
                                  name=f"rsum_{p0}")
                nc.vector.reciprocal(rsum[:], sums[:])
                for g in range(2):
                    nc.vector.tensor_scalar_mul(probs[:, g, :], probs[:, g, :],
                                                rsum[:, g:g + 1])
                dst = out.rearrange("(p g) c -> p g c", g=2)
                nc.sync.dma_start(dst[p0:p1], probs[:])

            def score_block(et, m0, blk, tag):
                """Energy GEMM + tanh + *v + h-sum + partition-reduce for
                tokens [m0, m0+blk); et chunk c holds d-tiles 2c, 2c+1 of
                exactly those tokens."""
                acc = apool.tile([128, blk], fp32, tag="acc",
                                 name=f"acc{tag}")
                prev_veng = None
                for ht in range(H_TILES):
                    pe = pe_pool.tile([128, blk], fp32, tag="epsum")
                    for c in range(D_TILES // 2):
                        nc.tensor.matmul(
                            pe[:], wt_sb[ht][:, 2 * c:2 * c + 2, :],
                            et[c][:, :, 0:blk],
                            start=(c == 0), stop=(c == D_TILES // 2 - 1),
                            perf_mode=DR)
                    eng = gpool.tile([128, blk], fp32, tag="eng")
                    nc.scalar.activation(eng[:], pe[:], AF.Tanh,
                                         bias=b_sb[:, ht:ht + 1], scale=DEQ)
                    veng = vpool.tile([128, blk], fp32, tag="veng",
                                      name=f"veng{tag}_{ht}")
                    nc.scalar.mul(veng[:], eng[:], v_sb[:, ht:ht + 1])
                    # running accumulation: ready ~one ACT after the last MM
                    if ht == 1:
                        nc.vector.tensor_add(acc[:], prev_veng[:], veng[:])
                    elif ht > 1:
                        nc.vector.tensor_add(acc[:], acc[:], veng[:])
                    prev_veng = veng
                # scores[m] = sum over all 1024 h = partition-reduce of acc
                red = apool.tile([128, blk], fp32, tag="red",
                                 name=f"red{tag}")
                nc.gpsimd.partition_all_reduce(red[:], acc[:], 128,
                                               bass_isa.ReduceOp.add)
                nc.sync.dma_start(sc_dram[:, m0:m0 + blk], red[0:1, :])

            v_bf = cpool.tile([128, H_TILES], bf16)
            nc.vector.tensor_copy(v_bf[:], v_sb[:])

            def tail_block(et, m0, blk):
                """Last tokens: scores via M=1 bf16 matmuls (deferred one
                h-tile so the PE never waits on ScalarE) and an inline
                single-partition softmax — a much shorter critical chain
                than the gpsimd/DRAM-bounce path."""
                nl = blk // B  # l rows covered
                sps = ps1pool.tile([1, blk], fp32, tag="sps")
                engs = []
                for ht in range(H_TILES):
                    pe = pe_pool.tile([128, blk], fp32, tag="epsum")
                    for c in range(D_TILES // 2):
                        nc.tensor.matmul(
                            pe[:], wt_sb[ht][:, 2 * c:2 * c + 2, :],
                            et[c][:, :, 0:blk],
                            start=(c == 0), stop=(c == D_TILES // 2 - 1),
                            perf_mode=DR)
                    eng = gpool.tile([128, blk], bf16, tag="engbf",
                                     name=f"engbf{ht}")
                    nc.scalar.activation(eng[:], pe[:], AF.Tanh,
                                         bias=b_sb[:, ht:ht + 1], scale=DEQ)
                    engs.append(eng)
                    # defer the score matvec two h-tiles so it never waits
                    # on the ScalarE queue
                    if ht >= 2:
                        nc.tensor.matmul(sps[:], v_bf[:, ht - 2:ht - 1],
                                         engs[ht - 2][:], start=(ht == 2),
                                         stop=False)
                for ht in (H_TILES - 2, H_TILES - 1):
                    nc.tensor.matmul(sps[:], v_bf[:, ht:ht + 1],
                                     engs[ht][:], start=False,
                                     stop=(ht == H_TILES - 1))
                st = mpool.tile([1, nl, B], fp32, tag="st")
                nc.scalar.activation(st[:], sps.rearrange("o (l c) -> o l c",
                                                          c=B), AF.Exp)
                tsum = mpool.tile([1, nl], fp32, tag="tsum")
                nc.vector.reduce_sum(tsum[:], st[:],
                                     axis=mybir.AxisListType.X)
                trs = mpool.tile([1, nl], fp32, tag="trs")
                nc.vector.reciprocal(trs[:], tsum[:])
                nc.vector.tensor_tensor(st[:], st[:],
                                        trs[:, :, None].to_broadcast(st.shape),
                                        mybir.AluOpType.mult)
                l0 = m0 // B
                dst = out.rearrange("(a l) c -> a l c", l=nl)
                nc.sync.dma_start(dst[l0 // nl:l0 // nl + 1], st[:])

            # Partitions 96..123 normalize after mb30 (covered by mb31's
            # score half + the tail block); only the tiny (124,126) range
            # waits on the last gpsimd/DRAM bounce, hidden under the tail
            # block's matmuls. The kernel ends on the short-chain tail.
            TAIL = 256
            for mb in range(N_BLKS - 1):
                et = (et0 if mb == 0 else
                      et1 if mb == 1 else load_et(mb))
                score_block(et, mb * M_BLK, M_BLK, str(mb))
                if mb == 7:
                    softmax_range(0, 32)
                elif mb == 15:
                    softmax_range(32, 64)
                elif mb == 23:
                    softmax_range(64, 96)
                elif mb == 30:
                    softmax_range(96, 124)
            et31a = load_et(31, tok0=0, ntok=M_BLK - TAIL)
            score_block(et31a, 31 * M_BLK, M_BLK - TAIL, "31a")
            softmax_range(124, 126)
            et31b = load_et(31, tok0=M_BLK - TAIL, ntok=TAIL)
            tail_block(et31b, 31 * M_BLK + (M_BLK - TAIL), TAIL)

    nc.compile()
    return nc


def kernel(num_features, encoder_outputs, W, b, v):
    global LAST_RESULTS
    from concourse.bass_utils import run_bass_kernel_spmd

    enc = np.asarray(encoder_outputs, dtype=np.float32)
    W_np = np.asarray(W, dtype=np.float32)
    b_np = np.asarray(b, dtype=np.float32)
    v_np = np.asarray(v, dtype=np.float32)
    F = int(np.asarray(num_features))
    assert enc.shape == (L, B, D) and W_np.shape == (H, D)

    # wr[ht, k, dt, j] = W[ht*128 + j, dt*128 + k], quantized to e4m3 at
    # x512 — contiguous 2KB per SBUF partition for a single clean DMA.
    wr_np = np.clip(
        W_np.reshape(H_TILES, 128, D_TILES, 128).transpose(0, 3, 2, 1)
        * SCALE_W, -240.0, 240.0).astype(E4M3)
    wr_np = np.ascontiguousarray(wr_np)
    bT_np = np.ascontiguousarray(b_np.reshape(H_TILES, 128).T)     # [128, 8]
    vT_np = np.ascontiguousarray(v_np.ravel().reshape(H_TILES, 128).T)

    in_maps = []
    for c in range(N_CORES):
        shard = np.clip(
            enc[c * L_LOC:(c + 1) * L_LOC].reshape(M, D) * SCALE_E,
            -240.0, 240.0).astype(E4M3)
        encT_np = np.ascontiguousarray(shard.T)                    # [D, M]
        in_maps.append({"encT": encT_np, "wr": wr_np, "bT": bT_np,
                        "vT": vT_np})

    if "nc" not in _compiled:
        _compiled["nc"] = _build()
    nc = _compiled["nc"]

    res = run_bass_kernel_spmd(nc, in_maps, core_ids=list(range(N_CORES)))
    LAST_RESULTS = res

    probs = np.concatenate([res.results[c]["out"] for c in range(N_CORES)],
                           axis=0)                                 # [L, B]
    out = np.broadcast_to(probs.T[:, None, :], (B, F, L))
    return np.ascontiguousarray(out)

